# revision 1
# baseline (speedup 1.0000x reference)
"""LMS adaptive filter (BaseFilter) on 8 TRN2 NeuronCores.

Algorithm per (batch b, frame f): 64-tap LMS over 416 sequential steps.
  e_t   = d[b, 256f + 32 + t] - sum_k w[k] * x[256f + t + k]
  w     = clip(w + MU * e_t * x[256f + t : +64], +-65535)
The clip is essential: mu*|x_win|^2 ~ 3.2 > 2 makes the recursion
unstable, so w rides the clip rails and the rails keep all float
implementations shadowing each other. f32 required (bf16 diverges).

Sharding: 4096 frames split 512/core (both batches on every core) ->
1024 independent sequences/core = 8 groups x 128 partitions.

Inner loop, per step per group (8 independent group chains round-robin):
  DVE   scalar_tensor_tensor : prod = (w * -1) * x_win ; ns = sum(prod)
  ScalarE activation Identity: e_t  = ns + d_t   (bias = per-partition d)
  DVE   scalar_tensor_tensor : w   += (mu*x_win) * e_t   (in-place)
  GpSimd tensor_scalar       : w    = clip(w, +-65535)
NS/PROD scratch tiles are double-buffered across steps (NBUF) to break
WAR hazards that otherwise serialize the accumulator pipeline.
(tensor_tensor_reduce crashes the exec unit on this runtime; custom
fused DVE ops measure +110-160ns/instr and lose; Pool cannot run
scalar_tensor_tensor; segmented axis reduce is Vector-only.)
"""

import numpy as np

HOP = 256
FRAMELEN = 512
K = 64
WD = 32
MU = 0.05
WMIN, WMAX = -65535.0, 65535.0
B = 2
F = 4096
NC = 8
F_LOC = F // NC              # 512 frames per core
S = (FRAMELEN - K) - WD      # 416 sequential steps
TSTART = (FRAMELEN - HOP) - WD  # 224: first step kept for frames >= 1
TAIL = S - TSTART            # 192 output elements per frame >= 1
SPAN = HOP * (F_LOC - 1) + FRAMELEN  # 131328: x/d elements per core shard
CORE_STRIDE = HOP * F_LOC    # 131072
OUT_LEN = (FRAMELEN - K) + (F - 1) * TAIL  # 786688

NBUF = 2                     # NS/PROD double-buffer depth
# groups whose update runs on Pool (tensor_mul + tensor_add); rest on DVE
POOL_UPD_GROUPS = ()

_CACHE = {}


def _custom_ops():
    """Register the two fused DVE ops (kept for microbenches; the kernel
    itself no longer uses them — measured +110-160ns/instr overhead)."""
    import concourse.dve_ops as dve_ops
    from concourse.dve_ops import DveOp
    from concourse.dve_spec import (
        Spec, Src0, Src1, C0, C1, C2, Zero, scan, AluOp as DveAluOp,
        minn, maxx, lower, _has_src1,
    )
    from concourse.dve_uop import DveOpSpec

    def _ref_scan_dot(in0, in1, s0, s1, imm2):
        c = np.cumsum(-(in0.astype(np.float32) * in1.astype(np.float32)),
                      axis=-1)
        return (np.asarray(s0).reshape(-1, 1) + c).astype(np.float32)

    def _ref_updclip(in0, in1, s0, s1, imm2):
        v = (in0.astype(np.float32) * np.asarray(s0).reshape(-1, 1)
             + in1.astype(np.float32))
        return np.clip(v, imm2, s1).astype(np.float32)

    def _register(name, spec, subdim=False):
        for op in dve_ops.OPS:
            if op.name == name:
                return op
        shas = {}
        for ver in ("v3", "v4"):
            tmp = DveOpSpec(name=name, opcode=1, uops=lower(spec, ver=ver),
                            rd1_en=_has_src1(spec))
            shas[ver] = tmp.sha(ver)
        op = DveOp(name, spec, subdim=subdim, uops_sha=shas)
        dve_ops.OPS.append(op)
        dve_ops.CUSTOM_DVE_SPECS[name] = spec
        dve_ops._SUB_OPCODE_FOR_NAME[name] = (
            dve_ops._CUSTOM_DVE_ROW_BASE + len(dve_ops.OPS) - 1)
        return op

    scan_dot = _register(
        "LMS_SCAN_DOT",
        Spec(body=scan(DveAluOp.ADD, Zero - Src0 * Src1, init=C0),
             reference=_ref_scan_dot),
    )
    updclip = _register(
        "LMS_UPD_CLIP",
        Spec(body=maxx(minn(Src0 * C0 + Src1, C1), C2),
             reference=_ref_updclip),
    )
    return scan_dot, updclip


def _build():
    import concourse.bacc as bacc
    import concourse.tile as tile
    from concourse import mybir
    import concourse.bass as bass

    f32 = mybir.dt.float32
    AluOp = mybir.AluOpType

    nc = bacc.Bacc("TRN2", target_bir_lowering=False)
    x_in = nc.dram_tensor("x", [SPAN], f32, kind="ExternalInput")
    d_in = nc.dram_tensor("d", [B, SPAN], f32, kind="ExternalInput")
    # [kind(0=d_est,1=e)][b][f_local][j] , j <-> step t = TSTART + j
    out_main = nc.dram_tensor("out_main", [2, B, F_LOC, TAIL], f32,
                              kind="ExternalOutput")
    # frame 0 of this core: steps t < TSTART   [kind][b][t]
    out_head = nc.dram_tensor("out_head", [2, B, TSTART], f32,
                              kind="ExternalOutput")

    with tile.TileContext(nc) as tc:
        with tc.tile_pool(name="p", bufs=1) as pool:
            XF = pool.tile([128, 4, FRAMELEN], f32)    # x frames (slab fg)
            XFMU = pool.tile([128, 4, FRAMELEN], f32)  # MU * x frames
            DB = pool.tile([128, B, 4, S], f32)        # d at step offsets
            W = [[pool.tile([128, K], f32, name=f"W{g}_{i}", tag=f"w{g}_{i}")
                  for i in range(2)] for g in range(8)]
            EB = [pool.tile([128, S], f32, name=f"EB{g}", tag=f"e{g}")
                  for g in range(8)]
            NS = [[pool.tile([128, 1], f32, name=f"NS{g}_{i}",
                             tag=f"n{g}_{i}") for i in range(NBUF)]
                  for g in range(8)]
            PROD = [[pool.tile([128, K], f32, name=f"PROD{g}_{i}",
                               tag=f"p{g}_{i}") for i in range(NBUF)]
                    for g in range(8)]
            XE = {g: pool.tile([128, K], f32, name=f"XE{g}", tag=f"x{g}")
                  for g in POOL_UPD_GROUPS}
            DEST = pool.tile([128, 8, S], f32, name="DEST", tag="dest")

            # partition p, slab fg  ->  frame f_local = fg*128 + p
            # Inputs split per slab so the first-step dots start as soon as
            # slab 0 lands instead of waiting on the whole 1MB load
            # (head measured 19.2us -> 12.5us).
            for g in range(8):
                nc.vector.memset(W[g][0][:], 0.0)
            for fg in range(4):
                nc.sync.dma_start(
                    XF[:, fg, :],
                    bass.AP(tensor=x_in, offset=HOP * 128 * fg,
                            ap=[[HOP, 128], [1, FRAMELEN]]),
                )
                nc.vector.tensor_scalar_mul(XFMU[:, fg, :], XF[:, fg, :], MU)
                for b in range(B):
                    nc.sync.dma_start(
                        DB[:, b, fg, :],
                        bass.AP(tensor=d_in,
                                offset=b * SPAN + HOP * 128 * fg + WD,
                                ap=[[HOP, 128], [1, S]]),
                    )

            def ebc(g, t):
                ap = EB[g][:]
                return bass.AP(tensor=ap.tensor, offset=ap.offset + t,
                               ap=[list(ap.ap[0]), [0, K]])

            def emit_dot(t, i, g):
                b, fg = divmod(g, 4)
                nc.vector.scalar_tensor_tensor(
                    out=PROD[g][i][:], in0=W[g][t % 2][:], scalar=-1.0,
                    in1=XF[:, fg, t:t + K],
                    op0=AluOp.mult, op1=AluOp.mult,
                    accum_out=NS[g][i][:, 0:1],
                )

            def emit_upd(t, g):
                b, fg = divmod(g, 4)
                cur, nxt = W[g][t % 2], W[g][(t + 1) % 2]
                if g in POOL_UPD_GROUPS:
                    nc.gpsimd.tensor_tensor(
                        out=XE[g][:], in0=XFMU[:, fg, t:t + K],
                        in1=ebc(g, t), op=AluOp.mult,
                    )
                    nc.gpsimd.tensor_tensor(
                        out=nxt[:], in0=cur[:],
                        in1=XE[g][:], op=AluOp.add,
                    )
                else:
                    nc.vector.scalar_tensor_tensor(
                        out=nxt[:], in0=XFMU[:, fg, t:t + K],
                        scalar=EB[g][:, t:t + 1], in1=cur[:],
                        op0=AluOp.mult, op1=AluOp.add,
                    )
                nc.gpsimd.tensor_scalar(
                    out=nxt[:], in0=nxt[:],
                    scalar1=WMAX, scalar2=WMIN,
                    op0=AluOp.min, op1=AluOp.max,
                )

            # 8 independent group chains, round-robin per step. DVE issue
            # order interleaves the tail dots with the head updates so the
            # accumulator-hazard waits after each dot arrive pre-satisfied
            # (they fuse instead of standing alone) and e-latency is hidden.
            def emit_act(t, i, g):
                b, fg = divmod(g, 4)
                nc.scalar.activation(
                    out=EB[g][:, t:t + 1], in_=NS[g][i][:, 0:1],
                    func=mybir.ActivationFunctionType.Identity,
                    bias=DB[:, b, fg, t:t + 1], scale=1.0,
                )

            for t in range(S):
                i = t % NBUF
                for g in range(8):
                    emit_dot(t, i, g)
                for g in range(8):
                    emit_act(t, i, g)
                for g in range(8):
                    emit_upd(t, g)

            # d_est = d - e  (single DEST tile -> batched output DMAs)
            for g in range(8):
                b, fg = divmod(g, 4)
                nc.vector.tensor_sub(DEST[:, g, :], DB[:, b, fg, :],
                                     EB[g][:])

            dap = DEST[:]
            # d_est main: one DMA for all 8 groups; g = b*4+fg page order
            nc.sync.dma_start(
                bass.AP(tensor=out_main, offset=0,
                        ap=[[TAIL, 128], [F_LOC * TAIL, B], [128 * TAIL, 4],
                            [1, TAIL]]),
                bass.AP(tensor=dap.tensor, offset=dap.offset + TSTART,
                        ap=[list(dap.ap[0]), [4 * S, B], [S, 4], [1, TAIL]]),
            )
            # d_est head: both batches in one DMA (pages 0 and 4, partition 0)
            nc.sync.dma_start(
                bass.AP(tensor=out_head, offset=0,
                        ap=[[TSTART, 1], [TSTART, B], [1, TSTART]]),
                bass.AP(tensor=dap.tensor, offset=dap.offset,
                        ap=[[dap.ap[0][0], 1], [4 * S, B], [1, TSTART]]),
            )
            # e outputs (EB remains per-group for chain-dep precision)
            for g in range(8):
                b, fg = divmod(g, 4)
                nc.sync.dma_start(
                    bass.AP(tensor=out_main,
                            offset=(B + b) * F_LOC * TAIL + fg * 128 * TAIL,
                            ap=[[TAIL, 128], [1, TAIL]]),
                    EB[g][:, TSTART:S],
                )
            for b in range(B):
                g = b * 4
                nc.sync.dma_start(
                    bass.AP(tensor=out_head,
                            offset=(B + b) * TSTART,
                            ap=[[TSTART, 1], [1, TSTART]]),
                    EB[g][0:1, 0:TSTART],
                )
    nc.finalize()
    return nc


def _get_nc():
    if "nc" not in _CACHE:
        _CACHE["nc"] = _build()
    return _CACHE["nc"]


def run_shards(d, x, trace=False, **kw):
    from concourse.bass_utils import run_bass_kernel_spmd

    nc = _get_nc()
    in_maps = []
    for c in range(NC):
        lo = c * CORE_STRIDE
        in_maps.append({
            "x": np.ascontiguousarray(x[lo:lo + SPAN], dtype=np.float32),
            "d": np.ascontiguousarray(d[:, lo:lo + SPAN], dtype=np.float32),
        })
    return run_bass_kernel_spmd(nc, in_maps, core_ids=list(range(NC)),
                                trace=trace, **kw)


def assemble(results):
    mains = np.stack([r["out_main"] for r in results])  # (8, 2, B, 512, 192)
    head = results[0]["out_head"]                       # (2, B, 224)
    outs = []
    for kind in range(2):
        m = mains[:, kind].transpose(1, 0, 2, 3).reshape(B, F, TAIL)
        o = np.zeros((B, OUT_LEN), np.float32)
        o[:, WD:WD + TSTART] = head[kind]
        o[:, WD + TSTART:FRAMELEN - K] = m[:, 0]
        o[:, FRAMELEN - K:] = m[:, 1:].reshape(B, -1)
        outs.append(o)
    return outs[0], outs[1]


def kernel(d, x):
    res = run_shards(d, x)
    return assemble(res.results)

